# revision 7
# baseline (speedup 1.0000x reference)
"""AdvancedFeatureTokenizer Trainium2 kernel.

Math (per batch row b, feature f):
  out[b,f,:] = mish(z[b,f,:]) @ W2p[f]            (tokens, proj-folded)
             + bin_term[bins[b,f]]                 (bin embedding, proj-folded)
             + pos_w[f] @ P2 + b2[f] @ P1 + proj_b (const per feature)
             + odt_term[b]                         (oblivious decision trees)

All per-feature weight folds are done host-side in float64 (O(weights) work).
On chip, per (feature, batch-chunk) a single K=96 matmul computes
tokens + bin + const:
  lhsT rows 0..63  = mish(z)  (z broadcast via PE outer-product matmuls)
  lhsT rows 64..94 = thermometer bits [x > edge_j]  (bin lookup as matmul)
  lhsT rows 95     = ones (const row)
The odt term is added during PSUM eviction.

Sharding: pure data parallel, batch 4096 -> 8 cores x 512.
"""

import os
from contextlib import ExitStack

import numpy as np

import concourse.bass as bass
import concourse.bacc as bacc
import concourse.mybir as mybir
import concourse.tile as tile
from concourse.bass_utils import run_bass_kernel_spmd

B, F, D, H, NB, T, L = 4096, 128, 256, 64, 32, 8, 8
D2, D4 = D // 2, D // 4
NCORES = 8
BL = B // NCORES          # 512 batch rows per core
NCHUNK = BL // 128        # 4 batch chunks of 128
GF = 8                    # features per weight/stage group
NGROUPS = F // GF         # 16
NPAIR_G = GF // 2         # 4 pairs per group
KDIM = H + 31 + 1         # 96 contraction rows

FP = mybir.dt.float32
F32R = mybir.dt.float32r  # replaced by MM_DT below
AL = mybir.AluOpType
AF = mybir.ActivationFunctionType

# matmul operand dtype: f32r (full-rate fp32-ish) | fp32 (4x slow) | bf16
MM_DT_NAME = os.environ.get("MM_DT", "f32r")
MM_DT = {"f32r": mybir.dt.float32r, "fp32": mybir.dt.float32,
         "bf16": mybir.dt.bfloat16}[MM_DT_NAME]
MM_NP = np.dtype("float32")
if MM_DT_NAME == "bf16":
    import ml_dtypes
    MM_NP = np.dtype(ml_dtypes.bfloat16)


def _mmdt(ap):
    return ap


def fold_inputs(inputs):
    """Host-side weight folding (float64, O(weights) only). Returns dict of
    per-core-replicated arrays + per-core x shards."""
    f8 = lambda a: np.asarray(a, np.float64)
    x = np.ascontiguousarray(np.asarray(inputs["x"], np.float32))
    w1 = np.asarray(inputs["w1"], np.float32)
    b1 = np.asarray(inputs["b1"], np.float32)
    w2 = f8(inputs["w2"])
    b2 = f8(inputs["b2"])
    pos_w = f8(inputs["pos_w"])
    bin_adjust = np.asarray(inputs["bin_adjust"], np.float32)
    bin_emb_w = f8(inputs["bin_emb_w"])
    sel_w = np.asarray(inputs["sel_w"], np.float32)
    sel_b = np.asarray(inputs["sel_b"], np.float32)
    leaf_w = f8(inputs["leaf_w"])
    proj_w = f8(inputs["proj_w"])
    proj_b = f8(inputs["proj_b"])

    P1 = proj_w[0:D2]                # tokens part
    P2 = proj_w[D2:D2 + D4]         # pos part
    P3 = proj_w[D2 + D4:D2 + 2 * D4]  # bin part
    P4 = proj_w[D2 + 2 * D4:]       # odt part

    w2p = np.einsum("fhd,de->fhe", w2, P1)             # [F, H, D]
    bin_term = bin_emb_w @ P3                           # [NB, D]
    delta = bin_term[1:] - bin_term[:-1]                # [NB-1, D]
    const = b2 @ P1 + pos_w @ P2 + bin_term[0] + proj_b  # [F, D]

    # lhsT row order: [mish (64) | therm (31) | ones (1)]
    wfull = np.concatenate(
        [w2p,
         np.broadcast_to(delta[None], (F, NB - 1, D)),
         const[:, None, :]],
        axis=1,
    )  # [F, 96, D]
    wfull_sb = np.ascontiguousarray(
        wfull.transpose(1, 0, 2).reshape(KDIM, F * D).astype(np.float32)
    )

    # edges: same fp32 arithmetic as the reference (linspace + bin_adjust)
    lin = np.linspace(-4.0, 4.0, NB + 1).astype(np.float32)
    edges_full = lin[None, :].astype(np.float32) + bin_adjust  # [F, NB+1] fp32
    edgesT = np.full((32, F), -3.0e38, np.float32)
    edgesT[0:31, :] = edges_full[:, 1:NB].T  # rows j=1..31; row 31 = -inf-ish (ones)

    # one-hot pair stationaries: moving operand is always full x_T[0:128]
    npair = F // 2
    zstat = np.zeros((npair, 128, 128), np.float32)
    tstat = np.zeros((npair, 128, 64), np.float32)
    for p in range(npair):
        f0, f1 = 2 * p, 2 * p + 1
        zstat[p, f0, 0:H] = w1[f0]
        zstat[p, f1, H:2 * H] = w1[f1]
        tstat[p, f0, 0:32] = 1.0
        tstat[p, f1, 32:64] = 1.0
    zstat_sb = np.ascontiguousarray(zstat.transpose(1, 0, 2).reshape(128, npair * 128))
    tstat_sb = np.ascontiguousarray(tstat.transpose(1, 0, 2).reshape(128, npair * 64))

    # b1 paired: partition h + 64*j <- b1[2p+j, h]
    b1p = np.ascontiguousarray(
        b1.reshape(F // 2, 2, H).transpose(1, 2, 0).reshape(2 * H, F // 2)
    )

    sel_flat = np.ascontiguousarray(sel_w.transpose(1, 0, 2).reshape(F, T * L))
    selb_b = np.ascontiguousarray(np.tile(sel_b.reshape(1, T * L), (128, 1)))

    leaf2p = (leaf_w.reshape(T * L, D4) @ P4) / T       # [64, D]
    leaf2p2 = np.ascontiguousarray(
        np.concatenate([leaf2p, leaf2p], axis=1).astype(np.float32)
    )  # [64, 2D]

    ident = np.eye(128, dtype=np.float32)

    rep = {
        "wfull": wfull_sb.astype(MM_NP),
        "zstat": zstat_sb.astype(MM_NP),
        "tstat": tstat_sb,
        "edgesT": edgesT,
        "b1p": b1p.astype(np.float32),
        "sel_flat": sel_flat.astype(np.float32),
        "selb_b": selb_b.astype(np.float32),
        "leaf2p2": leaf2p2,
        "ident": ident,
    }
    shards = [np.ascontiguousarray(x[c * BL:(c + 1) * BL]) for c in range(NCORES)]
    return rep, shards


def build_body(ctx: ExitStack, tc: "tile.TileContext", aps):
    nc = tc.nc
    x_d = aps["x"]
    out_d = aps["out"]

    consts = ctx.enter_context(tc.tile_pool(name="consts", bufs=1))
    x_T = consts.tile([128, BL], FP, tag="x_T")
    x_Tb = consts.tile([128, BL], MM_DT, tag="x_Tb")
    edgesT = consts.tile([32, F], FP, tag="edgesT")
    b1p = consts.tile([2 * H, F // 2], FP, tag="b1p")
    sel_flat = consts.tile([F, T * L], FP, tag="sel_flat")
    selb_b = consts.tile([128, T * L], FP, tag="selb_b")
    leaf2p2 = consts.tile([T * L, 2 * D], FP, tag="leaf2p2")
    ident = consts.tile([128, 128], FP, tag="ident")
    odt2 = [consts.tile([128, 2 * D], FP, tag=f"odt2_{i}", name=f"odt2_{i}")
            for i in range(NCHUNK)]

    nc.sync.dma_start(edgesT[:], aps["edgesT"])
    nc.sync.dma_start(b1p[:], aps["b1p"])
    nc.sync.dma_start(sel_flat[:], aps["sel_flat"])
    nc.sync.dma_start(selb_b[:], aps["selb_b"])
    nc.sync.dma_start(leaf2p2[:], aps["leaf2p2"])
    nc.sync.dma_start(ident[:], aps["ident"])

    spsum = ctx.enter_context(tc.tile_pool(name="spsum", bufs=1, space="PSUM"))
    swork = ctx.enter_context(tc.tile_pool(name="swork", bufs=2))

    # x load + transpose to [f, b]
    for i in range(NCHUNK):
        xb = swork.tile([128, 128], FP, tag="xb")
        nc.sync.dma_start(xb[:], x_d[i * 128:(i + 1) * 128, :])
        pt = spsum.tile([128, 512], FP, tag="sp")
        nc.tensor.transpose(pt[0:128, 0:128], xb[:], ident[:])
        nc.vector.tensor_copy(x_T[:, i * 128:(i + 1) * 128], pt[0:128, 0:128])
        nc.vector.tensor_copy(x_Tb[:, i * 128:(i + 1) * 128], pt[0:128, 0:128])

    # odt chain per batch chunk -> odt2[i] [128, 512] (d duplicated for pairs)
    for i in range(NCHUNK):
        pdec = spsum.tile([128, 512], FP, tag="sp")
        nc.tensor.matmul(pdec[0:128, 0:T * L], x_T[:, i * 128:(i + 1) * 128],
                         sel_flat[:])
        a = swork.tile([128, T * L], FP, tag="a")
        nc.vector.tensor_tensor(a[:], pdec[0:128, 0:T * L], selb_b[:], AL.add)
        th = swork.tile([128, T * L], FP, tag="th")
        # sigmoid(a) = (1 + tanh(a/2)) / 2 ; normalization cancels the 1/2
        nc.scalar.activation(th[:], a[:], AF.Tanh, scale=0.5)
        s8 = swork.tile([128, T], FP, tag="s8")
        nc.vector.tensor_reduce(s8[:], th[:].rearrange("p (t l) -> p t l", l=L),
                                mybir.AxisListType.X, AL.add)
        s8b = swork.tile([128, T], FP, tag="s8b")
        nc.vector.tensor_scalar_add(s8b[:], s8[:], float(L))
        rs = swork.tile([128, T], FP, tag="rs")
        nc.vector.reciprocal_approx_fast(rs[:], s8b[:])
        lp = swork.tile([128, T * L], FP, tag="lp")
        nc.vector.scalar_tensor_tensor(
            lp[:].rearrange("p (t l) -> p t l", l=L),
            th[:].rearrange("p (t l) -> p t l", l=L),
            1.0,
            rs[:].rearrange("p (t o) -> p t o", o=1).broadcast_to([128, T, L]),
            AL.add, AL.mult,
        )
        plt = spsum.tile([128, 512], FP, tag="sp")
        nc.tensor.transpose(plt[0:T * L, 0:128], lp[:], ident[:])
        lpt = swork.tile([T * L, 128], FP, tag="lpt")
        nc.vector.tensor_copy(lpt[:], plt[0:T * L, 0:128])
        podt = spsum.tile([128, 512], FP, tag="sp")
        nc.tensor.matmul(podt[0:128, 0:2 * D], _mmdt(lpt[:]), _mmdt(leaf2p2[:]))
        nc.vector.tensor_copy(odt2[i][:], podt[0:128, 0:2 * D])

    # main loop
    wpool = ctx.enter_context(tc.tile_pool(name="wpool", bufs=2))
    zpsum = ctx.enter_context(tc.tile_pool(name="zpsum", bufs=2, space="PSUM"))
    tpsum = ctx.enter_context(tc.tile_pool(name="tpsum", bufs=2, space="PSUM"))
    mpsum = ctx.enter_context(tc.tile_pool(name="mpsum", bufs=3, space="PSUM"))
    mwork = ctx.enter_context(tc.tile_pool(name="mwork", bufs=2))
    lpool = ctx.enter_context(tc.tile_pool(name="lhst", bufs=2))
    spool = ctx.enter_context(tc.tile_pool(name="stage", bufs=2))

    wfull_d = aps["wfull"]
    zstat_d = aps["zstat"]
    tstat_d = aps["tstat"]
    for g in range(NGROUPS):
        wg = wpool.tile([KDIM, GF * D], MM_DT, tag="wg")
        nc.sync.dma_start(wg[:], wfull_d[:, g * GF * D:(g + 1) * GF * D])
        zs = wpool.tile([128, NPAIR_G * 128], MM_DT, tag="zs")
        nc.sync.dma_start(zs[:], zstat_d[:, g * NPAIR_G * 128:(g + 1) * NPAIR_G * 128])
        ts = wpool.tile([128, NPAIR_G * 64], FP, tag="ts")
        nc.sync.dma_start(ts[:], tstat_d[:, g * NPAIR_G * 64:(g + 1) * NPAIR_G * 64])
        stages = [spool.tile([128, GF * D], FP, tag=f"st{c}", name=f"st{c}")
                  for c in range(NCHUNK)]
        for p in range(NPAIR_G):
            f0 = g * GF + 2 * p
            f1 = f0 + 1
            pg = g * NPAIR_G + p
            # z pre-activation via one-hot outer-product: rows 0-63 f0, 64-127 f1
            pz = zpsum.tile([128, BL], FP, tag="pz")
            nc.tensor.matmul(pz[:], _mmdt(zs[:, p * 128:(p + 1) * 128]),
                             _mmdt(x_Tb[:]))
            # x broadcast for thermometer (row 31 of each 32-block reserved
            # for the const/ones row: edge = -3e38 makes is_gt always 1)
            ptm = tpsum.tile([64, BL], FP, tag="ptm")
            nc.tensor.matmul(ptm[:], _mmdt(ts[:, p * 64:(p + 1) * 64]),
                             _mmdt(x_T[:]))
            # mish(z) = z * (1 - 2/((exp(z)+1)^2+1))
            E = mwork.tile([128, BL], FP, tag="E")
            nc.scalar.activation(E[:], pz[:], AF.Exp, bias=b1p[:, pg:pg + 1])
            W = mwork.tile([128, BL], FP, tag="W")
            nc.scalar.activation(W[:], E[:], AF.Square, bias=1.0)
            Dt = mwork.tile([128, BL], FP, tag="Dt")
            nc.vector.tensor_scalar_add(Dt[:], W[:], 1.0)
            R = mwork.tile([128, BL], FP, tag="R")
            nc.vector.reciprocal_approx_fast(R[:], Dt[:])
            Q = mwork.tile([128, BL], FP, tag="Q")
            nc.vector.tensor_scalar(Q[:], R[:], -2.0, 1.0, AL.mult, AL.add)
            Z = mwork.tile([128, BL], FP, tag="Z")
            nc.scalar.activation(Z[:], pz[:], AF.Identity, bias=b1p[:, pg:pg + 1])
            l0 = lpool.tile([KDIM, BL], MM_DT, tag="l0")
            l1 = lpool.tile([KDIM, BL], MM_DT, tag="l1")
            nc.gpsimd.tensor_tensor(l0[0:H, :], Z[0:H, :], Q[0:H, :], AL.mult)
            nc.gpsimd.tensor_tensor(l1[0:H, :], Z[H:2 * H, :], Q[H:2 * H, :], AL.mult)
            nc.vector.tensor_scalar(l0[H:KDIM, :], ptm[0:32, :],
                                    edgesT[:, f0:f0 + 1], None, AL.is_gt)
            nc.vector.tensor_scalar(l1[H:KDIM, :], ptm[32:64, :],
                                    edgesT[:, f1:f1 + 1], None, AL.is_gt)
            for c in range(NCHUNK):
                pm = mpsum.tile([128, 2 * D], FP, tag="pm")
                nc.tensor.matmul(pm[:, 0:D], _mmdt(l0[:, c * 128:(c + 1) * 128]),
                                 _mmdt(wg[:, (2 * p) * D:(2 * p + 1) * D]))
                nc.tensor.matmul(pm[:, D:2 * D], _mmdt(l1[:, c * 128:(c + 1) * 128]),
                                 _mmdt(wg[:, (2 * p + 1) * D:(2 * p + 2) * D]))
                nc.vector.tensor_tensor(stages[c][:, p * 2 * D:(p + 1) * 2 * D],
                                        pm[:], odt2[c][:], AL.add)
        for c in range(NCHUNK):
            nc.sync.dma_start(
                out_d[c * 128:(c + 1) * 128, g * GF:(g + 1) * GF, :],
                stages[c][:].rearrange("p (f d) -> p f d", d=D),
            )


def build_nc():
    nc = bacc.Bacc("TRN2", target_bir_lowering=False, debug=False)
    aps = {}
    aps["x"] = nc.dram_tensor("x", [BL, F], FP, kind="ExternalInput").ap()
    aps["wfull"] = nc.dram_tensor("wfull", [KDIM, F * D], MM_DT,
                                  kind="ExternalInput").ap()
    aps["zstat"] = nc.dram_tensor("zstat", [128, (F // 2) * 128], MM_DT,
                                  kind="ExternalInput").ap()
    aps["tstat"] = nc.dram_tensor("tstat", [128, (F // 2) * 64], FP,
                                  kind="ExternalInput").ap()
    aps["edgesT"] = nc.dram_tensor("edgesT", [32, F], FP,
                                   kind="ExternalInput").ap()
    aps["b1p"] = nc.dram_tensor("b1p", [2 * H, F // 2], FP,
                                kind="ExternalInput").ap()
    aps["sel_flat"] = nc.dram_tensor("sel_flat", [F, T * L], FP,
                                     kind="ExternalInput").ap()
    aps["selb_b"] = nc.dram_tensor("selb_b", [128, T * L], FP,
                                   kind="ExternalInput").ap()
    aps["leaf2p2"] = nc.dram_tensor("leaf2p2", [T * L, 2 * D], FP,
                                    kind="ExternalInput").ap()
    aps["ident"] = nc.dram_tensor("ident", [128, 128], FP,
                                  kind="ExternalInput").ap()
    aps["out"] = nc.dram_tensor("out", [BL, F, D], FP,
                                kind="ExternalOutput").ap()

    with tile.TileContext(nc) as tc, ExitStack() as ctx:
        build_body(ctx, tc, aps)
    nc.compile()
    return nc


def make_in_maps(inputs):
    rep, shards = fold_inputs(inputs)
    in_maps = []
    for c in range(NCORES):
        m = dict(rep)
        m["x"] = shards[c]
        in_maps.append(m)
    return in_maps


_NC_CACHE = {}


def get_nc():
    if "nc" not in _NC_CACHE:
        _NC_CACHE["nc"] = build_nc()
    return _NC_CACHE["nc"]


def kernel(**inputs) -> np.ndarray:
    nc = get_nc()
    in_maps = make_in_maps(inputs)
    res = run_bass_kernel_spmd(nc, in_maps, list(range(NCORES)))
    out = np.concatenate([res.results[c]["out"] for c in range(NCORES)], axis=0)
    return np.ascontiguousarray(out.reshape(B, F, D).astype(np.float32))
